# revision 4
# baseline (speedup 1.0000x reference)
"""GNN message-passing on 8 TRN2 NeuronCores — merged-pair descriptor gather.

The dma_gather path is SWDGE descriptor-rate bound (~2.2ns/desc at 4 queues;
512B descriptors cost only ~8%% more than 256B). So we merge TWO edges into
ONE 512B descriptor wherever possible, using a redundant partner-slot table:

  T[j] = [ A_j | B_j | C_j ]   (768B row; B_j = bf16 pair-row j = x[2j],x[2j+1];
                                A_j, C_j = freely chosen partner pair-rows)

A merged pair of same-bucket edges (u, v) reads EITHER
  bytes [j*768,     j*768+512) = [A_j|B_j]  with j=v, A_v:=u   ("AB" call), or
  bytes [j*768+256, j*768+768) = [B_j|C_j]  with j=u, C_u:=v   ("BC" call),
one 512B descriptor landing in two adjacent tile slots (same partition).
Unmerged edges use 256B descriptors at offset +256B ("S" call). All calls use
elem_step = 384 elems (768B row stride) on the same table.

Tiles remain (dst-block, src-parity)-pure; scatter is the same one-hot
(is_equal vs iota) matmul accumulating f32 PSUM per dst block. Tile schedule
groups same-call-type regions across super-buckets of blocks to amortize the
994ns/call SWDGE fixed cost while keeping <=12 PSUM tiles live.
"""

import numpy as np
import ml_dtypes

import concourse.tile as tile
from concourse import bacc, mybir
from concourse import bass_utils

N_NODES = 50000
D = 64
N_CORES = 8
NODES_PER_CORE = N_NODES // N_CORES
BLOCK = 128
N_PAIRS = N_NODES // 2
PAD_REL = 999.0
SB_BLOCKS = 6          # blocks per super-bucket (PSUM: 8 banks)
CALL_TILES = 16        # max tiles per gather call (<=1024 descriptors)


def assign_nodes(deg_ev, deg_od, n_cores, n_blocks, block):
    n_nodes = len(deg_ev)
    nb = n_cores * n_blocks
    ev_s = np.zeros(nb)
    od_s = np.zeros(nb)
    cnt = np.zeros(nb, np.int64)
    assign = np.empty(n_nodes, np.int64)
    order = np.argsort(-(deg_ev + deg_od), kind="stable")
    full_penalty = np.zeros(nb)
    for n in order:
        score = np.maximum(ev_s + deg_ev[n], od_s + deg_od[n]) + full_penalty
        b = int(np.argmin(score))
        assign[n] = b
        ev_s[b] += deg_ev[n]
        od_s[b] += deg_od[n]
        cnt[b] += 1
        if cnt[b] >= block:
            full_penalty[b] = 1e18
    core_of = assign // n_blocks
    block_of = assign % n_blocks
    pos_of = np.empty(n_nodes, np.int64)
    fill = np.zeros(nb, np.int64)
    for n in order:
        b = assign[n]
        pos_of[n] = fill[b]
        fill[b] += 1
    return core_of, block_of, pos_of


def pair_core_edges(buckets, n_pairs=N_PAIRS, bc_even=256, ab_even=128):
    """Pair up edges within buckets, assigning partner slots A/C per table row.

    Host-scan greedy: an edge whose row has a free C (or A) slot can host a
    merged pair with ANY same-bucket partner. Round-robin one pair per bucket
    per round so slot capacity (25000 each for A/C) is consumed evenly.
    Per-bucket targets alternate (bc_even, ab_even) / (ab_even, bc_even).

    Returns dict (blk, par) -> {"ab": [(j, rel_first, rel_second)...],
    "bc": [...], "s": [(row, rel)...]}, plus slot arrays A_val, C_val.

    Merged desc covers (first_edge@tile t, second_edge@tile t+1):
      BC @ j: [B_j|C_j] -> (j-edge, C_val[j]-edge)
      AB @ j: [A_j|B_j] -> (A_val[j]-edge, j-edge)
    """
    A_val = np.full(n_pairs, -1, np.int64)
    C_val = np.full(n_pairs, -1, np.int64)
    keys = sorted(buckets.keys())
    res = {}
    tgt = {}
    pools = {}
    for i, key in enumerate(keys):
        edges = sorted(buckets[key])
        pools[key] = edges
        res[key] = {"ab": [], "bc": [], "s": []}
        tgt[key] = ((bc_even, ab_even) if i % 2 == 0
                    else (ab_even, bc_even))  # (bc_t, ab_t)

    def run_phase(slot_val, kind, tidx):
        # one merged pair per bucket per round, round-robin
        st = {key: {"i": 0, "nh": []} for key in keys}
        active = True
        while active:
            active = False
            for key in keys:
                lst = res[key][kind]
                if len(lst) >= tgt[key][tidx]:
                    continue
                s = st[key]
                pool = pools[key]
                made = False
                while s["i"] < len(pool):
                    q, r = pool[s["i"]]
                    s["i"] += 1
                    if slot_val[q] < 0:
                        # host found; partner: prefer a non-host edge
                        if s["nh"]:
                            pq, pr = s["nh"].pop()
                        elif s["i"] < len(pool):
                            pq, pr = pool[s["i"]]
                            s["i"] += 1
                        else:
                            s["nh"].append((q, r))
                            break
                        slot_val[q] = pq
                        if kind == "bc":
                            lst.append((q, r, pr))     # (j, first, second)
                        else:
                            lst.append((q, pr, r))    # first=partner, second=j
                        made = True
                        break
                    else:
                        s["nh"].append((q, r))
                if made:
                    active = True
        # leftovers for next phase
        for key in keys:
            s = st[key]
            pools[key] = s["nh"] + pools[key][s["i"]:]

    run_phase(C_val, "bc", 0)
    run_phase(A_val, "ab", 1)
    for key in keys:
        res[key]["s"] = pools[key]
    return res, A_val, C_val


def bin_edges2(edge_index, n_cores=N_CORES, nodes_per_core=NODES_PER_CORE,
               block=BLOCK):
    dst = np.asarray(edge_index[0], dtype=np.int64)
    src = np.asarray(edge_index[1], dtype=np.int64)
    n_nodes = n_cores * nodes_per_core
    n_blocks = -(-nodes_per_core // block) + 1  # 50

    par = src & 1
    pair = src >> 1
    deg_ev = np.bincount(dst[par == 0], minlength=n_nodes)
    deg_od = np.bincount(dst[par == 1], minlength=n_nodes)
    core_of, block_of, pos_of = assign_nodes(
        deg_ev, deg_od, n_cores, n_blocks, block
    )

    core = core_of[dst]
    blk = block_of[dst]
    rel = pos_of[dst]

    # per-core pairing
    per_core = []
    for c in range(n_cores):
        mask = core == c
        buckets = {}
        for b, p, q, r in zip(blk[mask], par[mask], pair[mask], rel[mask]):
            buckets.setdefault((int(b), int(p)), []).append((int(q), int(r)))
        paired, A_val, C_val = pair_core_edges(buckets)
        per_core.append((paired, A_val, C_val))

    # shared plan: per (blk, par) -> n_ab, n_bc pair-cols and n_s single tiles
    n_ab = np.zeros((n_blocks, 2), np.int64)
    n_bc = np.zeros((n_blocks, 2), np.int64)
    n_s = np.zeros((n_blocks, 2), np.int64)
    for b in range(n_blocks):
        for p in range(2):
            abm = min(len(per_core[c][0].get((b, p), {"ab": []})["ab"])
                      for c in range(n_cores))
            bcm = min(len(per_core[c][0].get((b, p), {"bc": []})["bc"])
                      for c in range(n_cores))
            n_ab[b, p] = abm // 128
            n_bc[b, p] = bcm // 128
            # singles after demotions of excess pairs
            smax = 0
            for c in range(n_cores):
                bk = per_core[c][0].get((b, p), {"ab": [], "bc": [], "s": []})
                s = len(bk["s"])
                s += 2 * max(0, len(bk["ab"]) - n_ab[b, p] * 128)
                s += 2 * max(0, len(bk["bc"]) - n_bc[b, p] * 128)
                smax = max(smax, s)
            n_s[b, p] = max(1, -(-smax // 128)) if (smax or
                (n_ab[b, p] + n_bc[b, p] == 0)) else 0

    # ---- global tile schedule (shared across cores) ----
    # super-buckets of SB_BLOCKS blocks; regions [AB | BC | S], each split
    # into calls of <= CALL_TILES tiles.
    # tiles: list of (blk, par, kind, k) kind in {"ab","bc","s"}; pair-cols
    # contribute 2 tiles each (kind tagged on first tile of the pair-col).
    calls = []   # (kind, tile_start, n_tiles, ndesc)
    tile_blk = []
    tile_par = []
    abs_t = 0

    def emit_region(kind, cols):
        # cols: list of (blk, par) one entry per pair-col (2 tiles) or
        # single-tile (1 tile)
        nonlocal abs_t
        i = 0
        while i < len(cols):
            step = (CALL_TILES // 2) if kind != "s" else (CALL_TILES // 2)
            take = cols[i : i + step]
            tpc = 2 if kind != "s" else 1
            start = abs_t
            for (b, p) in take:
                for _ in range(tpc):
                    tile_blk.append(b)
                    tile_par.append(p)
                    abs_t += 1
            calls.append((kind, start, len(take) * tpc, len(take) * 128))
            i += step

    for sb0 in range(0, n_blocks, SB_BLOCKS):
        sbl = range(sb0, min(sb0 + SB_BLOCKS, n_blocks))
        ab_cols, bc_cols, s_cols = [], [], []
        for b in sbl:
            for p in range(2):
                ab_cols += [(b, p)] * int(n_ab[b, p])
                bc_cols += [(b, p)] * int(n_bc[b, p])
                s_cols += [(b, p)] * int(n_s[b, p])
        emit_region("ab", ab_cols)
        emit_region("bc", bc_cols)
        emit_region("s", s_cols)

    tot_tiles = abs_t
    tile_blk = np.array(tile_blk)
    tile_par = np.array(tile_par)
    # first/last tile of each block in global order
    first_of_blk = {}
    last_of_blk = {}
    for t in range(tot_tiles):
        b = int(tile_blk[t])
        if b not in first_of_blk:
            first_of_blk[b] = t
        last_of_blk[b] = t

    # ---- per-core idx streams + rel + table ----
    tot_desc = sum(nd for (_, _, _, nd) in calls)
    src16 = np.zeros((n_cores, tot_desc), np.int16)
    rel_pad = np.full((n_cores, tot_tiles * 128), PAD_REL, np.float32)
    tables = []
    for c in range(n_cores):
        paired, A_val, C_val = per_core[c]
        # build per-bucket streams with demotions
        use = {}
        for b in range(n_blocks):
            for p in range(2):
                bk = paired.get((b, p), {"ab": [], "bc": [], "s": []})
                na, nb_ = int(n_ab[b, p]) * 128, int(n_bc[b, p]) * 128
                ab = bk["ab"][:na]
                bc = bk["bc"][:nb_]
                s = list(bk["s"])
                for (j, r1, r2) in bk["ab"][na:]:
                    va = A_val[j]
                    s.append((int(va), r1))
                    s.append((int(j), r2))
                for (j, r1, r2) in bk["bc"][nb_:]:
                    vc = C_val[j]
                    s.append((int(j), r1))
                    s.append((int(vc), r2))
                s = s[: int(n_s[b, p]) * 128]
                use[(b, p)] = {"ab": ab, "bc": bc, "s": s}

        # fill calls
        dpos = 0
        tcur = {}
        for (kind, tstart, ntl, nd) in calls:
            # which (b,p) cols does this call cover? reconstruct from tiles
            t = tstart
            cols = []
            step = 2 if kind != "s" else 1
            for i in range(0, ntl, step):
                cols.append((int(tile_blk[t + i]), int(tile_par[t + i])))
            for ci, (b, p) in enumerate(cols):
                lst = use[(b, p)][kind]
                k0 = tcur.setdefault((b, p, kind), 0)
                items = lst[k0 : k0 + 128]
                tcur[(b, p, kind)] = k0 + 128
                base_t = tstart + ci * step
                if kind == "s":
                    for sl, (row, r) in enumerate(items):
                        src16[c, dpos + ci * 128 + sl] = row
                        rel_pad[c, (base_t) * 128 + sl] = r
                    if items:
                        # pad idx slots repeat last row
                        for sl in range(len(items), 128):
                            src16[c, dpos + ci * 128 + sl] = items[-1][0]
                else:
                    for sl, (j, r1, r2) in enumerate(items):
                        src16[c, dpos + ci * 128 + sl] = j
                        rel_pad[c, base_t * 128 + sl] = r1
                        rel_pad[c, (base_t + 1) * 128 + sl] = r2
                    if items:
                        for sl in range(len(items), 128):
                            src16[c, dpos + ci * 128 + sl] = items[0][0]
            dpos += nd
        assert dpos == tot_desc

        tables.append((A_val.copy(), C_val.copy()))

    # wrap idx streams to the [128, tot_desc//16] layout
    w = src16.reshape(n_cores, -1, 16).transpose(0, 2, 1)
    src16w = np.tile(w, (1, 8, 1)).copy()

    dstrel = (
        rel_pad.reshape(n_cores, -1, 128)
        .transpose(0, 2, 1)
        .astype(ml_dtypes.bfloat16)
        .copy()
    )

    meta = dict(
        calls=calls, tot_tiles=tot_tiles, tot_desc=tot_desc,
        tile_blk=tile_blk, tile_par=tile_par,
        first_of_blk=first_of_blk, last_of_blk=last_of_blk,
        n_blocks=n_blocks,
    )
    return meta, src16w, dstrel, tables, (core_of, block_of, pos_of)


def make_table(x, A_val, C_val):
    """[N_PAIRS, 384] bf16: row j = [pair A_val[j] | pair j | pair C_val[j]]."""
    xb = np.asarray(x, np.float32).astype(ml_dtypes.bfloat16)
    xpr = xb.reshape(N_PAIRS, 2 * D)
    T = np.empty((N_PAIRS, 6 * D), ml_dtypes.bfloat16)
    a = np.where(A_val < 0, np.arange(N_PAIRS), A_val)
    c = np.where(C_val < 0, np.arange(N_PAIRS), C_val)
    T[:, 0 : 2 * D] = xpr[a]
    T[:, 2 * D : 4 * D] = xpr
    T[:, 4 * D : 6 * D] = xpr[c]
    return T


def make_iota():
    return np.broadcast_to(
        np.arange(BLOCK, dtype=np.float32)[None, :], (128, BLOCK)
    ).astype(ml_dtypes.bfloat16).copy()


def build_program(meta, repeat=1, msgs_bufs=14, sel_bufs=14, psum_bufs=8,
                  n_queues=4):
    calls = meta["calls"]
    tot_tiles = meta["tot_tiles"]
    tot_desc = meta["tot_desc"]
    tile_blk = meta["tile_blk"]
    tile_par = meta["tile_par"]
    first_of_blk = meta["first_of_blk"]
    last_of_blk = meta["last_of_blk"]
    n_blocks = meta["n_blocks"]
    d = D
    d2 = 2 * D       # elems per pair row slice
    d6 = 6 * D       # table row elems (384)
    out_cols = n_blocks * BLOCK

    nc = bacc.Bacc(
        "TRN2",
        target_bir_lowering=False,
        debug=False,
        num_devices=N_CORES,
        num_swdge_queues=4,
    )
    tbl = nc.dram_tensor("tbl", [N_PAIRS, d6], mybir.dt.bfloat16,
                         kind="ExternalInput")
    src16 = nc.dram_tensor("src16", [128, tot_desc // 16], mybir.dt.int16,
                           kind="ExternalInput")
    dstrel = nc.dram_tensor("dstrel", [128, tot_tiles], mybir.dt.bfloat16,
                            kind="ExternalInput")
    iota_in = nc.dram_tensor("iota", [128, BLOCK], mybir.dt.bfloat16,
                             kind="ExternalInput")
    out = nc.dram_tensor("out", [d, out_cols], mybir.dt.float32,
                         kind="ExternalOutput")

    with tile.TileContext(nc) as tc:
        with (
            tc.tile_pool(name="meta", bufs=1) as meta_pool,
            tc.tile_pool(name="msgs", bufs=msgs_bufs) as msgs_pool,
            tc.tile_pool(name="sel", bufs=sel_bufs) as sel_pool,
            tc.tile_pool(name="obuf", bufs=2) as obuf_pool,
            tc.tile_pool(name="psum", bufs=psum_bufs, space="PSUM") as psum_pool,
        ):
            src_t = meta_pool.tile([128, tot_desc // 16], mybir.dt.int16)
            nc.sync.dma_start(src_t[:], src16.ap())
            rel_t = meta_pool.tile([128, tot_tiles], mybir.dt.bfloat16)
            nc.sync.dma_start(rel_t[:], dstrel.ap())
            iota_t = meta_pool.tile([128, BLOCK], mybir.dt.bfloat16)
            nc.sync.dma_start(iota_t[:], iota_in.ap())

            def body():
                outbuf = obuf_pool.tile([d, out_cols], mybir.dt.float32,
                                        tag="ob")
                psums = {}
                q = 0
                dcol = 0  # idx column offset (desc/16 units)
                for (kind, tstart, ntl, ndesc) in calls:
                    msgs = msgs_pool.tile([128, CALL_TILES, d2],
                                          mybir.dt.bfloat16, tag="msgs")
                    idx_ap = src_t[:, dcol : dcol + ndesc // 16]
                    if kind == "s":
                        nc.gpsimd.dma_gather(
                            msgs[:, 0:ntl, :],
                            tbl.ap()[:, d2 : 2 * d2],
                            idx_ap, ndesc, ndesc, d2,
                            elem_step=d6,
                            queue_num=q % n_queues,
                            single_packet=False,
                        )
                    else:
                        off = 0 if kind == "ab" else d2
                        nc.gpsimd.dma_gather(
                            msgs[:, 0:ntl, :].rearrange(
                                "p (o two) e -> p o (two e)", two=2
                            ),
                            tbl.ap()[:, off : off + 2 * d2],
                            idx_ap, ndesc, ndesc, 2 * d2,
                            elem_step=d6,
                            queue_num=q % n_queues,
                            single_packet=False,
                        )
                    q += 1
                    dcol += ndesc // 16

                    sel = sel_pool.tile([128, CALL_TILES, BLOCK],
                                        mybir.dt.bfloat16, tag="sel")
                    nc.vector.tensor_tensor(
                        out=sel[:, 0:ntl, :],
                        in0=rel_t[:, tstart : tstart + ntl].to_broadcast(
                            [128, ntl, BLOCK]
                        ),
                        in1=iota_t[:]
                        .rearrange("p (o n) -> p o n", o=1)
                        .to_broadcast([128, ntl, BLOCK]),
                        op=mybir.AluOpType.is_equal,
                    )

                    for i in range(ntl):
                        t = tstart + i
                        b = int(tile_blk[t])
                        parity = int(tile_par[t])
                        if first_of_blk[b] == t:
                            psums[b] = psum_pool.tile(
                                [d, BLOCK], mybir.dt.float32, space="PSUM",
                                tag="ps", name="ps",
                            )
                        nc.tensor.matmul(
                            out=psums[b][:],
                            lhsT=msgs[:, i, parity * d : (parity + 1) * d],
                            rhs=sel[:, i, :],
                            start=(first_of_blk[b] == t),
                            stop=(last_of_blk[b] == t),
                        )
                        if last_of_blk[b] == t:
                            nc.scalar.mul(
                                outbuf[:, b * BLOCK : (b + 1) * BLOCK],
                                psums[b][:],
                                1.0,
                            )
                            del psums[b]
                nc.sync.dma_start(out.ap(), outbuf[:])

            if repeat > 1:
                with tc.For_i(0, repeat, 1):
                    body()
            else:
                body()

    nc.compile()
    return nc


def unshard_output(results, node_loc, block=BLOCK, n_nodes=N_NODES, d=D):
    core_of, block_of, pos_of = node_loc
    cols = block_of * block + pos_of
    out = np.empty((n_nodes, d), dtype=np.float32)
    for c in range(len(results)):
        mask = core_of == c
        out[mask] = results[c]["out"].T[cols[mask]]
    return out


def prep_inputs(inputs):
    edge_index = np.asarray(inputs["edge_index"])
    x = np.ascontiguousarray(np.asarray(inputs["x"], np.float32))
    meta, src16w, dstrel, tables, node_loc = bin_edges2(edge_index)
    iota = make_iota()
    in_maps = []
    for c in range(N_CORES):
        A_val, C_val = tables[c]
        in_maps.append({
            "tbl": make_table(x, A_val, C_val),
            "src16": src16w[c],
            "dstrel": dstrel[c],
            "iota": iota,
        })
    return (meta,), in_maps, node_loc


def build(build_args, repeat=1):
    (meta,) = build_args
    return build_program(meta, repeat=repeat)


def kernel(edge_index, x):
    build_args, in_maps, node_loc = prep_inputs(
        {"edge_index": edge_index, "x": x}
    )
    nc = build(build_args)
    res = bass_utils.run_bass_kernel_spmd(nc, in_maps,
                                          core_ids=list(range(N_CORES)))
    return unshard_output(res.results, node_loc)


# revision 5
# speedup vs baseline: 1.0607x; 1.0607x over previous
"""GNN message-passing on 8 TRN2 NeuronCores — merged-pair descriptor gather.

The dma_gather path is SWDGE descriptor-rate bound (~2.2ns/desc at 4 queues;
512B descriptors cost only ~8%% more than 256B). So we merge TWO edges into
ONE 512B descriptor wherever possible, using a redundant partner-slot table:

  T[j] = [ A_j | B_j | C_j ]   (768B row; B_j = bf16 pair-row j = x[2j],x[2j+1];
                                A_j, C_j = freely chosen partner pair-rows)

A merged pair of same-bucket edges (u, v) reads EITHER
  bytes [j*768,     j*768+512) = [A_j|B_j]  with j=v, A_v:=u   ("AB" call), or
  bytes [j*768+256, j*768+768) = [B_j|C_j]  with j=u, C_u:=v   ("BC" call),
one 512B descriptor landing in two adjacent tile slots (same partition).
Width-5 rows [A|B|C|F|G] (1280B) add a free-floating FG window @+768B whose
both slots are host-chosen, absorbing ALL remaining pairs (and lone edges as
half-padded pairs) -> ~100%% of edges ride 512B/2-edge descriptors. Rare
leftovers use 256B "S" descriptors at offset +256B. elem_step = 640 elems.

Tiles remain (dst-block, src-parity)-pure; scatter is the same one-hot
(is_equal vs iota) matmul accumulating f32 PSUM per dst block. Tile schedule
groups same-call-type regions across super-buckets of blocks to amortize the
994ns/call SWDGE fixed cost while keeping <=12 PSUM tiles live.
"""

import numpy as np
import ml_dtypes

import concourse.tile as tile
from concourse import bacc, mybir
from concourse import bass_utils

N_NODES = 50000
D = 64
N_CORES = 8
NODES_PER_CORE = N_NODES // N_CORES
BLOCK = 128
N_PAIRS = N_NODES // 2
PAD_REL = 999.0
SB_BLOCKS = 6          # blocks per super-bucket (PSUM: 8 banks)
CALL_TILES = 16        # max tiles per gather call (<=1024 descriptors)


def assign_nodes(deg_ev, deg_od, n_cores, n_blocks, block):
    n_nodes = len(deg_ev)
    nb = n_cores * n_blocks
    ev_s = np.zeros(nb)
    od_s = np.zeros(nb)
    cnt = np.zeros(nb, np.int64)
    assign = np.empty(n_nodes, np.int64)
    order = np.argsort(-(deg_ev + deg_od), kind="stable")
    full_penalty = np.zeros(nb)
    for n in order:
        score = np.maximum(ev_s + deg_ev[n], od_s + deg_od[n]) + full_penalty
        b = int(np.argmin(score))
        assign[n] = b
        ev_s[b] += deg_ev[n]
        od_s[b] += deg_od[n]
        cnt[b] += 1
        if cnt[b] >= block:
            full_penalty[b] = 1e18
    core_of = assign // n_blocks
    block_of = assign % n_blocks
    pos_of = np.empty(n_nodes, np.int64)
    fill = np.zeros(nb, np.int64)
    for n in order:
        b = assign[n]
        pos_of[n] = fill[b]
        fill[b] += 1
    return core_of, block_of, pos_of


def pair_core_edges(buckets, n_pairs=N_PAIRS, bc_even=256, ab_even=128):
    """Pair up edges within buckets, assigning partner slots A/C per table row.

    Host-scan greedy: an edge whose row has a free C (or A) slot can host a
    merged pair with ANY same-bucket partner. Round-robin one pair per bucket
    per round so slot capacity (25000 each for A/C) is consumed evenly.
    Per-bucket targets alternate (bc_even, ab_even) / (ab_even, bc_even).

    Returns dict (blk, par) -> {"ab": [(j, rel_first, rel_second)...],
    "bc": [...], "s": [(row, rel)...]}, plus slot arrays A_val, C_val.

    Merged desc covers (first_edge@tile t, second_edge@tile t+1):
      BC @ j: [B_j|C_j] -> (j-edge, C_val[j]-edge)
      AB @ j: [A_j|B_j] -> (A_val[j]-edge, j-edge)
    """
    A_val = np.full(n_pairs, -1, np.int64)
    C_val = np.full(n_pairs, -1, np.int64)
    keys = sorted(buckets.keys())
    res = {}
    tgt = {}
    pools = {}
    for i, key in enumerate(keys):
        edges = sorted(buckets[key])
        pools[key] = edges
        res[key] = {"ab": [], "bc": [], "rem": []}
        tgt[key] = ((bc_even, ab_even) if i % 2 == 0
                    else (ab_even, bc_even))  # (bc_t, ab_t)

    def run_phase(slot_val, kind, tidx):
        # one merged pair per bucket per round, round-robin
        st = {key: {"i": 0, "nh": []} for key in keys}
        active = True
        while active:
            active = False
            for key in keys:
                lst = res[key][kind]
                if len(lst) >= tgt[key][tidx]:
                    continue
                s = st[key]
                pool = pools[key]
                made = False
                while s["i"] < len(pool):
                    q, r = pool[s["i"]]
                    s["i"] += 1
                    if slot_val[q] < 0:
                        # host found; partner: prefer a non-host edge
                        if s["nh"]:
                            pq, pr = s["nh"].pop()
                        elif s["i"] < len(pool):
                            pq, pr = pool[s["i"]]
                            s["i"] += 1
                        else:
                            s["nh"].append((q, r))
                            break
                        slot_val[q] = pq
                        if kind == "bc":
                            lst.append((q, r, pr))     # (j, first, second)
                        else:
                            lst.append((q, pr, r))    # first=partner, second=j
                        made = True
                        break
                    else:
                        s["nh"].append((q, r))
                if made:
                    active = True
        # leftovers for next phase
        for key in keys:
            s = st[key]
            pools[key] = s["nh"] + pools[key][s["i"]:]

    run_phase(C_val, "bc", 0)
    run_phase(A_val, "ab", 1)
    for key in keys:
        res[key]["rem"] = pools[key]
    return res, A_val, C_val


def bin_edges2(edge_index, n_cores=N_CORES, nodes_per_core=NODES_PER_CORE,
               block=BLOCK):
    dst = np.asarray(edge_index[0], dtype=np.int64)
    src = np.asarray(edge_index[1], dtype=np.int64)
    n_nodes = n_cores * nodes_per_core
    n_blocks = -(-nodes_per_core // block) + 1  # 50

    par = src & 1
    pair = src >> 1
    deg_ev = np.bincount(dst[par == 0], minlength=n_nodes)
    deg_od = np.bincount(dst[par == 1], minlength=n_nodes)
    core_of, block_of, pos_of = assign_nodes(
        deg_ev, deg_od, n_cores, n_blocks, block
    )

    core = core_of[dst]
    blk = block_of[dst]
    rel = pos_of[dst]

    # per-core AB/BC pairing
    per_core = []
    for c in range(n_cores):
        mask = core == c
        buckets = {}
        for b, p, q, r in zip(blk[mask], par[mask], pair[mask], rel[mask]):
            buckets.setdefault((int(b), int(p)), []).append((int(q), int(r)))
        paired, A_val, C_val = pair_core_edges(buckets)
        per_core.append((paired, A_val, C_val))

    # shared plan for host-window cols
    n_ab = np.zeros((n_blocks, 2), np.int64)
    n_bc = np.zeros((n_blocks, 2), np.int64)
    for b in range(n_blocks):
        for p in range(2):
            abm = min(len(per_core[c][0].get((b, p), {"ab": []})["ab"])
                      for c in range(n_cores))
            bcm = min(len(per_core[c][0].get((b, p), {"bc": []})["bc"])
                      for c in range(n_cores))
            n_ab[b, p] = abm // 128
            n_bc[b, p] = bcm // 128

    # per-core FG pairing over remainder (incl. demoted AB/BC pairs);
    # lone edges become half-padded FG pairs (second slot rel=PAD).
    # fg entry: (fgrow, q1, r1, q2, r2)
    fg_tabs = []
    fg_all = []
    n_fg = np.zeros((n_blocks, 2), np.int64)
    for c in range(n_cores):
        paired, A_val, C_val = per_core[c]
        F_val = np.full(N_PAIRS, -1, np.int64)
        G_val = np.full(N_PAIRS, -1, np.int64)
        nxt = 0
        fg = {}
        for b in range(n_blocks):
            for p in range(2):
                bk = paired.get((b, p), {"ab": [], "bc": [], "rem": []})
                pool = list(bk["rem"])
                for (j, r1, r2) in bk["ab"][int(n_ab[b, p]) * 128:]:
                    pool.append((int(A_val[j]), r1))
                    pool.append((int(j), r2))
                for (j, r1, r2) in bk["bc"][int(n_bc[b, p]) * 128:]:
                    pool.append((int(j), r1))
                    pool.append((int(C_val[j]), r2))
                lst = []
                for i in range(0, len(pool) - 1, 2):
                    (u, ru), (v, rv) = pool[i], pool[i + 1]
                    F_val[nxt] = u
                    G_val[nxt] = v
                    lst.append((nxt, u, ru, v, rv))
                    nxt += 1
                if len(pool) % 2:
                    (u, ru) = pool[-1]
                    F_val[nxt] = u
                    G_val[nxt] = u
                    lst.append((nxt, u, ru, u, PAD_REL))
                    nxt += 1
                fg[(b, p)] = lst
        assert nxt <= N_PAIRS, f"FG rows exhausted: {nxt}"
        fg_all.append(fg)
        fg_tabs.append((F_val, G_val))
    for b in range(n_blocks):
        for p in range(2):
            fgm = max(len(fg_all[c].get((b, p), [])) for c in range(n_cores))
            n_fg[b, p] = -(-fgm // 128)

    # ---- global tile schedule (shared across cores) ----
    calls = []   # (kind, tile_start, n_tiles, ndesc)
    tile_blk = []
    tile_par = []
    abs_t = 0

    def emit_region(kind, cols):
        nonlocal abs_t
        i = 0
        while i < len(cols):
            step = CALL_TILES // 2
            take = cols[i : i + step]
            start = abs_t
            for (b, p) in take:
                for _ in range(2):
                    tile_blk.append(b)
                    tile_par.append(p)
                    abs_t += 1
            calls.append((kind, start, len(take) * 2, len(take) * 128))
            i += step

    for sb0 in range(0, n_blocks, SB_BLOCKS):
        sbl = range(sb0, min(sb0 + SB_BLOCKS, n_blocks))
        ab_cols, bc_cols, fg_cols = [], [], []
        for b in sbl:
            for p in range(2):
                ab_cols += [(b, p)] * int(n_ab[b, p])
                bc_cols += [(b, p)] * int(n_bc[b, p])
                fg_cols += [(b, p)] * int(n_fg[b, p])
        emit_region("ab", ab_cols)
        emit_region("bc", bc_cols)
        emit_region("fg", fg_cols)

    tot_tiles = abs_t
    tile_blk = np.array(tile_blk)
    tile_par = np.array(tile_par)
    first_of_blk = {}
    last_of_blk = {}
    for t in range(tot_tiles):
        b = int(tile_blk[t])
        if b not in first_of_blk:
            first_of_blk[b] = t
        last_of_blk[b] = t

    # ---- per-core idx streams + rel ----
    tot_desc = sum(nd for (_, _, _, nd) in calls)
    src16 = np.zeros((n_cores, tot_desc), np.int16)
    rel_pad = np.full((n_cores, tot_tiles * 128), PAD_REL, np.float32)
    tables = []
    for c in range(n_cores):
        paired, A_val, C_val = per_core[c]
        F_val, G_val = fg_tabs[c]
        use = {}
        for b in range(n_blocks):
            for p in range(2):
                bk = paired.get((b, p), {"ab": [], "bc": [], "rem": []})
                use[(b, p)] = {
                    "ab": bk["ab"][: int(n_ab[b, p]) * 128],
                    "bc": bk["bc"][: int(n_bc[b, p]) * 128],
                    "fg": [(row, r1, r2)
                           for (row, q1, r1, q2, r2) in fg_all[c].get((b, p), [])],
                }

        dpos = 0
        tcur = {}
        for (kind, tstart, ntl, nd) in calls:
            t = tstart
            cols = []
            for i in range(0, ntl, 2):
                cols.append((int(tile_blk[t + i]), int(tile_par[t + i])))
            for ci, (b, p) in enumerate(cols):
                lst = use[(b, p)][kind]
                k0 = tcur.setdefault((b, p, kind), 0)
                items = lst[k0 : k0 + 128]
                tcur[(b, p, kind)] = k0 + 128
                base_t = tstart + ci * 2
                for sl, (j, r1, r2) in enumerate(items):
                    src16[c, dpos + ci * 128 + sl] = j
                    rel_pad[c, base_t * 128 + sl] = r1
                    rel_pad[c, (base_t + 1) * 128 + sl] = r2
                if len(items) < 128:
                    padrow = items[0][0] if items else 0
                    for sl in range(len(items), 128):
                        src16[c, dpos + ci * 128 + sl] = padrow
            dpos += nd
        assert dpos == tot_desc

        tables.append((A_val.copy(), C_val.copy(), F_val.copy(), G_val.copy()))

    w = src16.reshape(n_cores, -1, 16).transpose(0, 2, 1)
    src16w = np.tile(w, (1, 8, 1)).copy()

    dstrel = (
        rel_pad.reshape(n_cores, -1, 128)
        .transpose(0, 2, 1)
        .astype(ml_dtypes.bfloat16)
        .copy()
    )

    meta = dict(
        calls=calls, tot_tiles=tot_tiles, tot_desc=tot_desc,
        tile_blk=tile_blk, tile_par=tile_par,
        first_of_blk=first_of_blk, last_of_blk=last_of_blk,
        n_blocks=n_blocks,
    )
    return meta, src16w, dstrel, tables, (core_of, block_of, pos_of)


def make_table(x, A_val, C_val, F_val, G_val):
    """Three dense [N_PAIRS, 256]-elem bf16 tables (512B rows, contiguous):
    T_ab[j] = [A_j | B_j], T_bc[j] = [B_j | C_j], T_fg[j] = [F_j | G_j]."""
    xb = np.asarray(x, np.float32).astype(ml_dtypes.bfloat16)
    xpr = xb.reshape(N_PAIRS, 2 * D)
    idx = np.arange(N_PAIRS)
    a = np.where(A_val < 0, idx, A_val)
    c = np.where(C_val < 0, idx, C_val)
    f = np.where(F_val < 0, idx, F_val)
    g = np.where(G_val < 0, idx, G_val)
    T_ab = np.concatenate([xpr[a], xpr], axis=1).copy()
    T_bc = np.concatenate([xpr, xpr[c]], axis=1).copy()
    T_fg = np.concatenate([xpr[f], xpr[g]], axis=1).copy()
    return T_ab, T_bc, T_fg


def make_iota():
    return np.broadcast_to(
        np.arange(BLOCK, dtype=np.float32)[None, :], (128, BLOCK)
    ).astype(ml_dtypes.bfloat16).copy()


def build_program(meta, repeat=1, msgs_bufs=18, sel_bufs=10, psum_bufs=8,
                  n_queues=4, gather_only=False):
    calls = meta["calls"]
    tot_tiles = meta["tot_tiles"]
    tot_desc = meta["tot_desc"]
    tile_blk = meta["tile_blk"]
    tile_par = meta["tile_par"]
    first_of_blk = meta["first_of_blk"]
    last_of_blk = meta["last_of_blk"]
    n_blocks = meta["n_blocks"]
    d = D
    d2 = 2 * D       # elems per pair row slice
    out_cols = n_blocks * BLOCK

    nc = bacc.Bacc(
        "TRN2",
        target_bir_lowering=False,
        debug=False,
        num_devices=N_CORES,
        num_swdge_queues=4,
    )
    tabs = {
        k: nc.dram_tensor(f"tbl_{k}", [N_PAIRS, 2 * d2], mybir.dt.bfloat16,
                          kind="ExternalInput")
        for k in ("ab", "bc", "fg")
    }
    src16 = nc.dram_tensor("src16", [128, tot_desc // 16], mybir.dt.int16,
                           kind="ExternalInput")
    dstrel = nc.dram_tensor("dstrel", [128, tot_tiles], mybir.dt.bfloat16,
                            kind="ExternalInput")
    iota_in = nc.dram_tensor("iota", [128, BLOCK], mybir.dt.bfloat16,
                             kind="ExternalInput")
    out = nc.dram_tensor("out", [d, out_cols], mybir.dt.float32,
                         kind="ExternalOutput")

    with tile.TileContext(nc) as tc:
        with (
            tc.tile_pool(name="meta", bufs=1) as meta_pool,
            tc.tile_pool(name="msgs", bufs=msgs_bufs) as msgs_pool,
            tc.tile_pool(name="sel", bufs=sel_bufs) as sel_pool,
            tc.tile_pool(name="obuf", bufs=2) as obuf_pool,
            tc.tile_pool(name="psum", bufs=psum_bufs, space="PSUM") as psum_pool,
        ):
            src_t = meta_pool.tile([128, tot_desc // 16], mybir.dt.int16)
            nc.sync.dma_start(src_t[:], src16.ap())
            rel_t = meta_pool.tile([128, tot_tiles], mybir.dt.bfloat16)
            nc.sync.dma_start(rel_t[:], dstrel.ap())
            iota_t = meta_pool.tile([128, BLOCK], mybir.dt.bfloat16)
            nc.sync.dma_start(iota_t[:], iota_in.ap())

            def body():
                outbuf = obuf_pool.tile([d, out_cols], mybir.dt.float32,
                                        tag="ob")
                if gather_only:
                    nc.vector.memset(outbuf[:], 0.0)
                psums = {}
                q = 0
                dcol = 0  # idx column offset (desc/16 units)
                for (kind, tstart, ntl, ndesc) in calls:
                    msgs = msgs_pool.tile([128, CALL_TILES, d2],
                                          mybir.dt.bfloat16, tag="msgs")
                    idx_ap = src_t[:, dcol : dcol + ndesc // 16]
                    nc.gpsimd.dma_gather(
                        msgs[:, 0:ntl, :].rearrange(
                            "p (o two) e -> p o (two e)", two=2
                        ),
                        tabs[kind].ap(),
                        idx_ap, ndesc, ndesc, 2 * d2,
                        queue_num=q % n_queues,
                        single_packet=False,
                    )
                    q += 1
                    dcol += ndesc // 16

                    if gather_only:
                        continue
                    sel = sel_pool.tile([128, CALL_TILES, BLOCK],
                                        mybir.dt.bfloat16, tag="sel")
                    nc.vector.tensor_tensor(
                        out=sel[:, 0:ntl, :],
                        in0=rel_t[:, tstart : tstart + ntl].to_broadcast(
                            [128, ntl, BLOCK]
                        ),
                        in1=iota_t[:]
                        .rearrange("p (o n) -> p o n", o=1)
                        .to_broadcast([128, ntl, BLOCK]),
                        op=mybir.AluOpType.is_equal,
                    )

                    for i in range(ntl):
                        t = tstart + i
                        b = int(tile_blk[t])
                        parity = int(tile_par[t])
                        if first_of_blk[b] == t:
                            psums[b] = psum_pool.tile(
                                [d, BLOCK], mybir.dt.float32, space="PSUM",
                                tag="ps", name="ps",
                            )
                        nc.tensor.matmul(
                            out=psums[b][:],
                            lhsT=msgs[:, i, parity * d : (parity + 1) * d],
                            rhs=sel[:, i, :],
                            start=(first_of_blk[b] == t),
                            stop=(last_of_blk[b] == t),
                        )
                        if last_of_blk[b] == t:
                            nc.scalar.mul(
                                outbuf[:, b * BLOCK : (b + 1) * BLOCK],
                                psums[b][:],
                                1.0,
                            )
                            del psums[b]
                nc.sync.dma_start(out.ap(), outbuf[:])

            if repeat > 1:
                with tc.For_i(0, repeat, 1):
                    body()
            else:
                body()

    nc.compile()
    return nc


def unshard_output(results, node_loc, block=BLOCK, n_nodes=N_NODES, d=D):
    core_of, block_of, pos_of = node_loc
    cols = block_of * block + pos_of
    out = np.empty((n_nodes, d), dtype=np.float32)
    for c in range(len(results)):
        mask = core_of == c
        out[mask] = results[c]["out"].T[cols[mask]]
    return out


def prep_inputs(inputs):
    edge_index = np.asarray(inputs["edge_index"])
    x = np.ascontiguousarray(np.asarray(inputs["x"], np.float32))
    meta, src16w, dstrel, tables, node_loc = bin_edges2(edge_index)
    iota = make_iota()
    in_maps = []
    for c in range(N_CORES):
        A_val, C_val, F_val, G_val = tables[c]
        T_ab, T_bc, T_fg = make_table(x, A_val, C_val, F_val, G_val)
        in_maps.append({
            "tbl_ab": T_ab,
            "tbl_bc": T_bc,
            "tbl_fg": T_fg,
            "src16": src16w[c],
            "dstrel": dstrel[c],
            "iota": iota,
        })
    return (meta,), in_maps, node_loc


def build(build_args, repeat=1):
    (meta,) = build_args
    return build_program(meta, repeat=repeat)


def kernel(edge_index, x):
    build_args, in_maps, node_loc = prep_inputs(
        {"edge_index": edge_index, "x": x}
    )
    nc = build(build_args)
    res = bass_utils.run_bass_kernel_spmd(nc, in_maps,
                                          core_ids=list(range(N_CORES)))
    return unshard_output(res.results, node_loc)


# revision 6
# speedup vs baseline: 1.2472x; 1.1758x over previous
"""GNN message-passing on 8 TRN2 NeuronCores — merged-pair descriptor gather.

The dma_gather path is SWDGE descriptor-rate bound (~2.2ns/desc at 4 queues;
512B descriptors cost only ~8%% more than 256B). So we merge TWO edges into
ONE 512B descriptor wherever possible, using a redundant partner-slot table:

  T[j] = [ A_j | B_j | C_j ]   (768B row; B_j = bf16 pair-row j = x[2j],x[2j+1];
                                A_j, C_j = freely chosen partner pair-rows)

A merged pair of same-bucket edges (u, v) reads EITHER
  bytes [j*768,     j*768+512) = [A_j|B_j]  with j=v, A_v:=u   ("AB" call), or
  bytes [j*768+256, j*768+768) = [B_j|C_j]  with j=u, C_u:=v   ("BC" call),
one 512B descriptor landing in two adjacent tile slots (same partition).
Width-5 rows [A|B|C|F|G] (1280B) add a free-floating FG window @+768B whose
both slots are host-chosen, absorbing ALL remaining pairs (and lone edges as
half-padded pairs) -> ~100%% of edges ride 512B/2-edge descriptors. Rare
leftovers use 256B "S" descriptors at offset +256B. elem_step = 640 elems.

Tiles remain (dst-block, src-parity)-pure; scatter is the same one-hot
(is_equal vs iota) matmul accumulating f32 PSUM per dst block. Tile schedule
groups same-call-type regions across super-buckets of blocks to amortize the
994ns/call SWDGE fixed cost while keeping <=12 PSUM tiles live.
"""

import numpy as np
import ml_dtypes

import concourse.tile as tile
from concourse import bacc, mybir
from concourse import bass_utils

N_NODES = 50000
D = 64
N_CORES = 8
NODES_PER_CORE = N_NODES // N_CORES
BLOCK = 128
N_PAIRS = N_NODES // 2
PAD_REL = 999.0
SB_BLOCKS = 6          # blocks per super-bucket (PSUM: 8 banks)
CALL_TILES = 16        # max tiles per gather call (<=1024 descriptors)


def assign_nodes(deg_ev, deg_od, n_cores, n_blocks, block):
    n_nodes = len(deg_ev)
    nb = n_cores * n_blocks
    ev_s = np.zeros(nb)
    od_s = np.zeros(nb)
    cnt = np.zeros(nb, np.int64)
    assign = np.empty(n_nodes, np.int64)
    order = np.argsort(-(deg_ev + deg_od), kind="stable")
    full_penalty = np.zeros(nb)
    for n in order:
        score = np.maximum(ev_s + deg_ev[n], od_s + deg_od[n]) + full_penalty
        b = int(np.argmin(score))
        assign[n] = b
        ev_s[b] += deg_ev[n]
        od_s[b] += deg_od[n]
        cnt[b] += 1
        if cnt[b] >= block:
            full_penalty[b] = 1e18
    core_of = assign // n_blocks
    block_of = assign % n_blocks
    pos_of = np.empty(n_nodes, np.int64)
    fill = np.zeros(nb, np.int64)
    for n in order:
        b = assign[n]
        pos_of[n] = fill[b]
        fill[b] += 1
    return core_of, block_of, pos_of


def pair_core_edges(buckets, n_pairs=N_PAIRS, bc_even=256, ab_even=128):
    """Pair up edges within buckets, assigning partner slots A/C per table row.

    Host-scan greedy: an edge whose row has a free C (or A) slot can host a
    merged pair with ANY same-bucket partner. Round-robin one pair per bucket
    per round so slot capacity (25000 each for A/C) is consumed evenly.
    Per-bucket targets alternate (bc_even, ab_even) / (ab_even, bc_even).

    Returns dict (blk, par) -> {"ab": [(j, rel_first, rel_second)...],
    "bc": [...], "s": [(row, rel)...]}, plus slot arrays A_val, C_val.

    Merged desc covers (first_edge@tile t, second_edge@tile t+1):
      BC @ j: [B_j|C_j] -> (j-edge, C_val[j]-edge)
      AB @ j: [A_j|B_j] -> (A_val[j]-edge, j-edge)
    """
    A_val = np.full(n_pairs, -1, np.int64)
    C_val = np.full(n_pairs, -1, np.int64)
    keys = sorted(buckets.keys())
    res = {}
    tgt = {}
    pools = {}
    for i, key in enumerate(keys):
        edges = sorted(buckets[key])
        pools[key] = edges
        res[key] = {"ab": [], "bc": [], "rem": []}
        tgt[key] = ((bc_even, ab_even) if i % 2 == 0
                    else (ab_even, bc_even))  # (bc_t, ab_t)

    def run_phase(slot_val, kind, tidx):
        # one merged pair per bucket per round, round-robin
        st = {key: {"i": 0, "nh": []} for key in keys}
        active = True
        while active:
            active = False
            for key in keys:
                lst = res[key][kind]
                if len(lst) >= tgt[key][tidx]:
                    continue
                s = st[key]
                pool = pools[key]
                made = False
                while s["i"] < len(pool):
                    q, r = pool[s["i"]]
                    s["i"] += 1
                    if slot_val[q] < 0:
                        # host found; partner: prefer a non-host edge
                        if s["nh"]:
                            pq, pr = s["nh"].pop()
                        elif s["i"] < len(pool):
                            pq, pr = pool[s["i"]]
                            s["i"] += 1
                        else:
                            s["nh"].append((q, r))
                            break
                        slot_val[q] = pq
                        if kind == "bc":
                            lst.append((q, r, pr))     # (j, first, second)
                        else:
                            lst.append((q, pr, r))    # first=partner, second=j
                        made = True
                        break
                    else:
                        s["nh"].append((q, r))
                if made:
                    active = True
        # leftovers for next phase
        for key in keys:
            s = st[key]
            pools[key] = s["nh"] + pools[key][s["i"]:]

    run_phase(C_val, "bc", 0)
    run_phase(A_val, "ab", 1)
    for key in keys:
        res[key]["rem"] = pools[key]
    return res, A_val, C_val


def bin_edges2(edge_index, n_cores=N_CORES, nodes_per_core=NODES_PER_CORE,
               block=BLOCK):
    dst = np.asarray(edge_index[0], dtype=np.int64)
    src = np.asarray(edge_index[1], dtype=np.int64)
    n_nodes = n_cores * nodes_per_core
    n_blocks = -(-nodes_per_core // block) + 1  # 50

    par = src & 1
    pair = src >> 1
    deg_ev = np.bincount(dst[par == 0], minlength=n_nodes)
    deg_od = np.bincount(dst[par == 1], minlength=n_nodes)
    core_of, block_of, pos_of = assign_nodes(
        deg_ev, deg_od, n_cores, n_blocks, block
    )

    core = core_of[dst]
    blk = block_of[dst]
    rel = pos_of[dst]

    # per-core AB/BC pairing
    per_core = []
    for c in range(n_cores):
        mask = core == c
        buckets = {}
        for b, p, q, r in zip(blk[mask], par[mask], pair[mask], rel[mask]):
            buckets.setdefault((int(b), int(p)), []).append((int(q), int(r)))
        paired, A_val, C_val = pair_core_edges(buckets)
        per_core.append((paired, A_val, C_val))

    # shared plan for host-window cols
    n_ab = np.zeros((n_blocks, 2), np.int64)
    n_bc = np.zeros((n_blocks, 2), np.int64)
    for b in range(n_blocks):
        for p in range(2):
            abm = min(len(per_core[c][0].get((b, p), {"ab": []})["ab"])
                      for c in range(n_cores))
            bcm = min(len(per_core[c][0].get((b, p), {"bc": []})["bc"])
                      for c in range(n_cores))
            n_ab[b, p] = abm // 128
            n_bc[b, p] = bcm // 128

    # per-core FG pairing over remainder (incl. demoted AB/BC pairs);
    # lone edges become half-padded FG pairs (second slot rel=PAD).
    # fg entry: (fgrow, q1, r1, q2, r2)
    fg_tabs = []
    fg_all = []
    n_fg = np.zeros((n_blocks, 2), np.int64)
    for c in range(n_cores):
        paired, A_val, C_val = per_core[c]
        F_val = np.full(N_PAIRS, -1, np.int64)
        G_val = np.full(N_PAIRS, -1, np.int64)
        nxt = 0
        fg = {}
        for b in range(n_blocks):
            for p in range(2):
                bk = paired.get((b, p), {"ab": [], "bc": [], "rem": []})
                pool = list(bk["rem"])
                for (j, r1, r2) in bk["ab"][int(n_ab[b, p]) * 128:]:
                    pool.append((int(A_val[j]), r1))
                    pool.append((int(j), r2))
                for (j, r1, r2) in bk["bc"][int(n_bc[b, p]) * 128:]:
                    pool.append((int(j), r1))
                    pool.append((int(C_val[j]), r2))
                lst = []
                for i in range(0, len(pool) - 1, 2):
                    (u, ru), (v, rv) = pool[i], pool[i + 1]
                    F_val[nxt] = u
                    G_val[nxt] = v
                    lst.append((nxt, u, ru, v, rv))
                    nxt += 1
                if len(pool) % 2:
                    (u, ru) = pool[-1]
                    F_val[nxt] = u
                    G_val[nxt] = u
                    lst.append((nxt, u, ru, u, PAD_REL))
                    nxt += 1
                fg[(b, p)] = lst
        assert nxt <= N_PAIRS, f"FG rows exhausted: {nxt}"
        fg_all.append(fg)
        fg_tabs.append((F_val, G_val))
    for b in range(n_blocks):
        for p in range(2):
            fgm = max(len(fg_all[c].get((b, p), [])) for c in range(n_cores))
            n_fg[b, p] = -(-fgm // 128)

    # ---- global tile schedule (shared across cores) ----
    calls = []   # (kind, tile_start, n_tiles, ndesc)
    tile_blk = []
    tile_par = []
    abs_t = 0

    def emit_region(kind, cols):
        nonlocal abs_t
        i = 0
        while i < len(cols):
            step = CALL_TILES // 2
            take = cols[i : i + step]
            start = abs_t
            for (b, p) in take:
                for _ in range(2):
                    tile_blk.append(b)
                    tile_par.append(p)
                    abs_t += 1
            calls.append((kind, start, len(take) * 2, len(take) * 128))
            i += step

    for sb0 in range(0, n_blocks, SB_BLOCKS):
        sbl = range(sb0, min(sb0 + SB_BLOCKS, n_blocks))
        ab_cols, bc_cols, fg_cols = [], [], []
        for b in sbl:
            for p in range(2):
                ab_cols += [(b, p)] * int(n_ab[b, p])
                bc_cols += [(b, p)] * int(n_bc[b, p])
                fg_cols += [(b, p)] * int(n_fg[b, p])
        emit_region("ab", ab_cols)
        emit_region("bc", bc_cols)
        emit_region("fg", fg_cols)

    tot_tiles = abs_t
    tile_blk = np.array(tile_blk)
    tile_par = np.array(tile_par)
    first_of_blk = {}
    last_of_blk = {}
    for t in range(tot_tiles):
        b = int(tile_blk[t])
        if b not in first_of_blk:
            first_of_blk[b] = t
        last_of_blk[b] = t

    # ---- per-core idx streams + rel ----
    tot_desc = sum(nd for (_, _, _, nd) in calls)
    src16 = np.zeros((n_cores, tot_desc), np.int16)
    rel_pad = np.full((n_cores, tot_tiles * 128), PAD_REL, np.float32)
    tables = []
    for c in range(n_cores):
        paired, A_val, C_val = per_core[c]
        F_val, G_val = fg_tabs[c]
        use = {}
        for b in range(n_blocks):
            for p in range(2):
                bk = paired.get((b, p), {"ab": [], "bc": [], "rem": []})
                use[(b, p)] = {
                    "ab": bk["ab"][: int(n_ab[b, p]) * 128],
                    "bc": bk["bc"][: int(n_bc[b, p]) * 128],
                    "fg": [(row, r1, r2)
                           for (row, q1, r1, q2, r2) in fg_all[c].get((b, p), [])],
                }

        dpos = 0
        tcur = {}
        for (kind, tstart, ntl, nd) in calls:
            t = tstart
            cols = []
            for i in range(0, ntl, 2):
                cols.append((int(tile_blk[t + i]), int(tile_par[t + i])))
            for ci, (b, p) in enumerate(cols):
                lst = use[(b, p)][kind]
                k0 = tcur.setdefault((b, p, kind), 0)
                items = lst[k0 : k0 + 128]
                tcur[(b, p, kind)] = k0 + 128
                base_t = tstart + ci * 2
                for sl, (j, r1, r2) in enumerate(items):
                    src16[c, dpos + ci * 128 + sl] = j
                    rel_pad[c, base_t * 128 + sl] = r1
                    rel_pad[c, (base_t + 1) * 128 + sl] = r2
                if len(items) < 128:
                    padrow = items[0][0] if items else 0
                    for sl in range(len(items), 128):
                        src16[c, dpos + ci * 128 + sl] = padrow
            dpos += nd
        assert dpos == tot_desc

        tables.append((A_val.copy(), C_val.copy(), F_val.copy(), G_val.copy()))

    w = src16.reshape(n_cores, -1, 16).transpose(0, 2, 1)
    src16w = np.tile(w, (1, 8, 1)).copy()

    dstrel = (
        rel_pad.reshape(n_cores, -1, 128)
        .transpose(0, 2, 1)
        .astype(ml_dtypes.bfloat16)
        .copy()
    )

    meta = dict(
        calls=calls, tot_tiles=tot_tiles, tot_desc=tot_desc,
        tile_blk=tile_blk, tile_par=tile_par,
        first_of_blk=first_of_blk, last_of_blk=last_of_blk,
        n_blocks=n_blocks,
    )
    return meta, src16w, dstrel, tables, (core_of, block_of, pos_of)


def make_table(x, A_val, C_val, F_val, G_val):
    """Three dense [N_PAIRS, 256]-elem bf16 tables (512B rows, contiguous):
    T_ab[j] = [A_j | B_j], T_bc[j] = [B_j | C_j], T_fg[j] = [F_j | G_j]."""
    xb = np.asarray(x, np.float32).astype(ml_dtypes.bfloat16)
    xpr = xb.reshape(N_PAIRS, 2 * D)
    idx = np.arange(N_PAIRS)
    a = np.where(A_val < 0, idx, A_val)
    c = np.where(C_val < 0, idx, C_val)
    f = np.where(F_val < 0, idx, F_val)
    g = np.where(G_val < 0, idx, G_val)
    T_ab = np.concatenate([xpr[a], xpr], axis=1).copy()
    T_bc = np.concatenate([xpr, xpr[c]], axis=1).copy()
    T_fg = np.concatenate([xpr[f], xpr[g]], axis=1).copy()
    return T_ab, T_bc, T_fg


def make_iota():
    return np.broadcast_to(
        np.arange(BLOCK, dtype=np.float32)[None, :], (128, BLOCK)
    ).astype(ml_dtypes.bfloat16).copy()


def build_program(meta, repeat=1, msgs_bufs=22, sel_bufs=8, psum_bufs=8,
                  n_queues=4, gather_only=False, scratch=16384,
                  balance_q=True):
    calls = meta["calls"]
    tot_tiles = meta["tot_tiles"]
    tot_desc = meta["tot_desc"]
    tile_blk = meta["tile_blk"]
    tile_par = meta["tile_par"]
    first_of_blk = meta["first_of_blk"]
    last_of_blk = meta["last_of_blk"]
    n_blocks = meta["n_blocks"]
    d = D
    d2 = 2 * D       # elems per pair row slice
    out_cols = n_blocks * BLOCK

    nc = bacc.Bacc(
        "TRN2",
        target_bir_lowering=False,
        debug=False,
        num_devices=N_CORES,
        num_swdge_queues=4,
        dynamic_dma_scratch_size=scratch,
    )
    tabs = {
        k: nc.dram_tensor(f"tbl_{k}", [N_PAIRS, 2 * d2], mybir.dt.bfloat16,
                          kind="ExternalInput")
        for k in ("ab", "bc", "fg")
    }
    src16 = nc.dram_tensor("src16", [128, tot_desc // 16], mybir.dt.int16,
                           kind="ExternalInput")
    dstrel = nc.dram_tensor("dstrel", [128, tot_tiles], mybir.dt.bfloat16,
                            kind="ExternalInput")
    iota_in = nc.dram_tensor("iota", [128, BLOCK], mybir.dt.bfloat16,
                             kind="ExternalInput")
    out = nc.dram_tensor("out", [d, out_cols], mybir.dt.float32,
                         kind="ExternalOutput")

    with tile.TileContext(nc) as tc:
        with (
            tc.tile_pool(name="meta", bufs=1) as meta_pool,
            tc.tile_pool(name="msgs", bufs=msgs_bufs) as msgs_pool,
            tc.tile_pool(name="sel", bufs=sel_bufs) as sel_pool,
            tc.tile_pool(name="obuf", bufs=2) as obuf_pool,
            tc.tile_pool(name="psum", bufs=psum_bufs, space="PSUM") as psum_pool,
        ):
            src_t = meta_pool.tile([128, tot_desc // 16], mybir.dt.int16)
            nc.sync.dma_start(src_t[:], src16.ap())
            rel_t = meta_pool.tile([128, tot_tiles], mybir.dt.bfloat16)
            nc.sync.dma_start(rel_t[:], dstrel.ap())
            iota_t = meta_pool.tile([128, BLOCK], mybir.dt.bfloat16)
            nc.sync.dma_start(iota_t[:], iota_in.ap())

            def body():
                outbuf = obuf_pool.tile([d, out_cols], mybir.dt.float32,
                                        tag="ob")
                if gather_only:
                    nc.vector.memset(outbuf[:], 0.0)
                psums = {}
                q = 0
                qload = [0] * n_queues
                dcol = 0  # idx column offset (desc/16 units)
                for (kind, tstart, ntl, ndesc) in calls:
                    msgs = msgs_pool.tile([128, CALL_TILES, d2],
                                          mybir.dt.bfloat16, tag="msgs")
                    idx_ap = src_t[:, dcol : dcol + ndesc // 16]
                    if balance_q:
                        qn = min(range(n_queues), key=lambda i: qload[i])
                        qload[qn] += ndesc
                    else:
                        qn = q % n_queues
                    nc.gpsimd.dma_gather(
                        msgs[:, 0:ntl, :].rearrange(
                            "p (o two) e -> p o (two e)", two=2
                        ),
                        tabs[kind].ap(),
                        idx_ap, ndesc, ndesc, 2 * d2,
                        queue_num=qn,
                        single_packet=False,
                    )
                    q += 1
                    dcol += ndesc // 16

                    if gather_only:
                        continue
                    sel = sel_pool.tile([128, CALL_TILES, BLOCK],
                                        mybir.dt.bfloat16, tag="sel")
                    nc.vector.tensor_tensor(
                        out=sel[:, 0:ntl, :],
                        in0=rel_t[:, tstart : tstart + ntl].to_broadcast(
                            [128, ntl, BLOCK]
                        ),
                        in1=iota_t[:]
                        .rearrange("p (o n) -> p o n", o=1)
                        .to_broadcast([128, ntl, BLOCK]),
                        op=mybir.AluOpType.is_equal,
                    )

                    for i in range(ntl):
                        t = tstart + i
                        b = int(tile_blk[t])
                        parity = int(tile_par[t])
                        if first_of_blk[b] == t:
                            psums[b] = psum_pool.tile(
                                [d, BLOCK], mybir.dt.float32, space="PSUM",
                                tag="ps", name="ps",
                            )
                        nc.tensor.matmul(
                            out=psums[b][:],
                            lhsT=msgs[:, i, parity * d : (parity + 1) * d],
                            rhs=sel[:, i, :],
                            start=(first_of_blk[b] == t),
                            stop=(last_of_blk[b] == t),
                        )
                        if last_of_blk[b] == t:
                            nc.scalar.mul(
                                outbuf[:, b * BLOCK : (b + 1) * BLOCK],
                                psums[b][:],
                                1.0,
                            )
                            del psums[b]
                nc.sync.dma_start(out.ap(), outbuf[:])

            if repeat > 1:
                with tc.For_i(0, repeat, 1):
                    body()
            else:
                body()

    nc.compile()
    return nc


def unshard_output(results, node_loc, block=BLOCK, n_nodes=N_NODES, d=D):
    core_of, block_of, pos_of = node_loc
    cols = block_of * block + pos_of
    out = np.empty((n_nodes, d), dtype=np.float32)
    for c in range(len(results)):
        mask = core_of == c
        out[mask] = results[c]["out"].T[cols[mask]]
    return out


def prep_inputs(inputs):
    edge_index = np.asarray(inputs["edge_index"])
    x = np.ascontiguousarray(np.asarray(inputs["x"], np.float32))
    meta, src16w, dstrel, tables, node_loc = bin_edges2(edge_index)
    iota = make_iota()
    in_maps = []
    for c in range(N_CORES):
        A_val, C_val, F_val, G_val = tables[c]
        T_ab, T_bc, T_fg = make_table(x, A_val, C_val, F_val, G_val)
        in_maps.append({
            "tbl_ab": T_ab,
            "tbl_bc": T_bc,
            "tbl_fg": T_fg,
            "src16": src16w[c],
            "dstrel": dstrel[c],
            "iota": iota,
        })
    return (meta,), in_maps, node_loc


def build(build_args, repeat=1):
    (meta,) = build_args
    return build_program(meta, repeat=repeat)


def kernel(edge_index, x):
    build_args, in_maps, node_loc = prep_inputs(
        {"edge_index": edge_index, "x": x}
    )
    nc = build(build_args)
    res = bass_utils.run_bass_kernel_spmd(nc, in_maps,
                                          core_ids=list(range(N_CORES)))
    return unshard_output(res.results, node_loc)
